# revision 1
# baseline (speedup 1.0000x reference)
"""Trainium2 Bass kernel for nn_DGN3 (causal top-K GNN message passing).

Problem (hardcoded from the reference):
    B=4, T=2048, D=256, K=8, R=3 rounds
    per round:  S = h @ h^T  (causal masked);  top-8 neighbors per row;
                msg = mean of selected h rows; blended = mix*h+(1-mix)*msg
                h  = mom*h + (1-mom)*gelu(blended*gain + bias)
    out = (h - x) * softplus-scale

Mapping: data-parallel over batch. 8 cores run the same program; core c
processes batch c % 4 (cores 4..7 duplicate, outputs ignored).

Numerics: scores and message matmuls run in bf16 hi/lo split form
(hi=bf16(h), lo=bf16(h-hi)); S = hi.hi^T+hi.lo^T+lo.hi^T gives ~2^-17
relative score error (selection is bit-identical to fp32 for this data,
verified by simulation). Top-8 selection via the DVE max8 instruction +
per-row threshold compare (counts are min(i+1,8) deterministically).
The mix*h blend term is folded into the message-matmul PSUM accumulation
with a diagonal stationary matrix.
"""

import json

import numpy as np
import ml_dtypes

import concourse.bass as bass
import concourse.mybir as mybir
from concourse.tile import TileContext, ScopedClock
from concourse.bass_utils import run_bass_kernel_spmd

# ---------------------------------------------------------------- constants
B, T, D, K, R = 4, 2048, 256, 8, 3
P = 128                 # partitions
NB = T // P             # 16 key/query blocks
KT = D // P             # 2 contraction tiles
NEG = -1e9
THRESH_FLOOR = -1e8     # t8' = max(t8, floor): handles rows with <8 causal
CHUNK = 512             # psum bank width (fp32)

f32 = mybir.dt.float32
bf16 = mybir.dt.bfloat16

MAX_WAITS = 1

# ------------------------------------------------------- walrus workarounds
# This walrus build allows very few semaphore waits per instruction.
# (a) split the Tile tail-drain's waits across SP NOPs;
# (b) post-process the BIR JSON moving excess waits onto same-engine NoOps.
_orig_to_json_bytes = bass.Bass.to_json_bytes


def _split_excess_waits(obj):
    n_fixed = 0
    if isinstance(obj, dict):
        for key, val in obj.items():
            if key == "instructions" and isinstance(val, list):
                new_list = []
                for inst in val:
                    si = inst.get("sync_info") if isinstance(inst, dict) else None
                    waits = si.get("on_wait") if si else None
                    if waits and len(waits) > MAX_WAITS:
                        extra = waits[: len(waits) - MAX_WAITS]
                        keep = waits[len(waits) - MAX_WAITS:]
                        for k in range(0, len(extra), MAX_WAITS):
                            n_fixed += 1
                            new_list.append({
                                "name": f"{inst['name']}-waitsplit{k}",
                                "opcode": "NoOp",
                                "engine": inst["engine"],
                                "ins": [],
                                "outs": [],
                                "debug": inst.get("debug"),
                                "sync_info": {
                                    "on_wait": extra[k: k + MAX_WAITS],
                                    "on_update": [],
                                },
                            })
                        si["on_wait"] = keep
                    new_list.append(inst)
                obj[key] = new_list
            else:
                n_fixed += _split_excess_waits(val)
    elif isinstance(obj, list):
        for val in obj:
            n_fixed += _split_excess_waits(val)
    return n_fixed


def _to_json_bytes_patched(self, *args, **kwargs):
    raw = _orig_to_json_bytes(self, *args, **kwargs)
    m = json.loads(raw)
    if _split_excess_waits(m) == 0:
        return raw
    return json.dumps(m).encode()


def _drain_and_barrier_split(self, tick_clock, wait_clock):
    nc = self.nc
    probe = nc.sync.nop()
    wait_clock.add_sem_waits(probe.ins, ScopedClock({None: tick_clock.global_clock}))
    si = probe.ins.sync_info
    if si is not None and len(si.on_wait) > 1:
        waits = list(si.on_wait)
        probe.ins.sync_info = mybir.SyncInfo(
            on_wait=waits[:1], on_update=list(si.on_update)
        )
        for w in waits[1:]:
            nop = nc.sync.nop()
            nop.ins.sync_info = mybir.SyncInfo(on_wait=[w], on_update=[])
    nc.sync.drain()
    nc.all_engine_barrier()
    popped = nc._tile_sem_poison_stack.pop()
    assert popped is self._sem_poison
    nc.clear_and_free_semaphores(list(self.sems.allocated().values()))
    nc.all_engine_barrier()


def _install_patches():
    TileContext._drain_and_barrier = _drain_and_barrier_split
    bass.Bass.to_json_bytes = _to_json_bytes_patched


_install_patches()


# ------------------------------------------------------------ host helpers
def _sigmoid(v):
    return 1.0 / (1.0 + np.exp(-np.float64(v)))


def _softplus(v):
    return np.log1p(np.exp(np.float64(v)))


def _hi_lo(a):
    hi = a.astype(ml_dtypes.bfloat16)
    lo = (a - hi.astype(np.float32)).astype(ml_dtypes.bfloat16)
    return hi, lo


# ------------------------------------------------------------ program build
def build_program(mix, momentum, scale, gain, bias, gelu_via_erf=False, n_reps=1):
    """Build the per-core Bass program (one batch of shape [T, D])."""
    nc = bass.Bass()
    x_d = nc.dram_tensor("x", [T, D], f32, kind="ExternalInput")
    out_d = nc.dram_tensor("out", [T, D], f32, kind="ExternalOutput")

    mix = [float(m) for m in mix]
    momentum = float(momentum)
    scale = float(scale)
    gain_is_one = np.allclose(gain, 1.0)
    bias_is_zero = np.allclose(bias, 0.0)

    # per-row neighbor counts (block 0 rows 0..6 have fewer than 8)
    c0 = np.minimum(np.arange(P) + 1, K).astype(np.float64)   # block 0
    c8 = np.full(P, float(K))                                 # blocks 1..15

    consts = {}
    alpha_lo_zero = {}
    for r in range(R):
        m = mix[r]
        alpha0 = (m / (1.0 - m)) * c0
        alpha8 = (m / (1.0 - m)) * c8
        consts[f"alpha0_hi_{r}"], consts[f"alpha0_lo_{r}"] = _hi_lo(
            np.diag(alpha0).astype(np.float32))
        consts[f"alpha8_hi_{r}"], consts[f"alpha8_lo_{r}"] = _hi_lo(
            np.diag(alpha8).astype(np.float32))
        alpha_lo_zero[r] = (not np.any(consts[f"alpha0_lo_{r}"])
                            and not np.any(consts[f"alpha8_lo_{r}"]))
        consts[f"svec0_{r}"] = ((1.0 - m) / c0).astype(np.float32)[:, None]
        consts[f"svec8_{r}"] = ((1.0 - m) / c8).astype(np.float32)[:, None]
    ii, jj = np.meshgrid(np.arange(P), np.arange(P), indexing="ij")
    consts["mask_diag"] = np.where(jj <= ii, 0.0, NEG).astype(np.float32)
    if not gain_is_one or not bias_is_zero:
        consts["gain_b"] = np.tile(np.asarray(gain, np.float32)[:, None, :],
                                   (1, P, 1))        # [R,P,D]
        consts["bias_b"] = np.tile(np.asarray(bias, np.float32)[:, None, :],
                                   (1, P, 1))

    handles = {k: nc.inline_tensor(np.ascontiguousarray(v), name=k)
               for k, v in consts.items()}

    with TileContext(nc) as tc:
        with tc.tile_pool(name="persist", bufs=1) as persist, \
             tc.tile_pool(name="work", bufs=2) as work, \
             tc.tile_pool(name="gtp", bufs=4) as gtp, \
             tc.tile_pool(name="ep", bufs=3) as ep, \
             tc.tile_pool(name="small", bufs=8) as small, \
             tc.tile_pool(name="psum_s", bufs=6, space="PSUM") as psum_s, \
             tc.tile_pool(name="psum_m", bufs=2, space="PSUM") as psum_m:

            # ---------------- persistent state
            h_nat = [persist.tile([P, NB, D], f32, name=f"h_nat{i}")
                     for i in range(3)]
            h_hilo = [persist.tile([P, NB, 2, D], bf16, name=f"h_hilo{i}")
                      for i in range(2)]
            hT_hilo = [persist.tile([P, 2 * KT, T], bf16, name=f"hT_hilo{i}")
                       for i in range(2)]

            cmask = persist.tile([P, P], f32, name="cmask")
            nc.sync.dma_start(cmask[:], handles["mask_diag"][:])
            cal = {}
            for r in range(R):
                for nm in (f"alpha0_hi_{r}", f"alpha0_lo_{r}",
                           f"alpha8_hi_{r}", f"alpha8_lo_{r}"):
                    tl = persist.tile([P, P], bf16, name=nm)
                    nc.sync.dma_start(tl[:], handles[nm][:])
                    cal[nm] = tl
                for nm in (f"svec0_{r}", f"svec8_{r}"):
                    tl = persist.tile([P, 1], f32, name=nm)
                    nc.sync.dma_start(tl[:], handles[nm][:])
                    cal[nm] = tl
            if not gain_is_one or not bias_is_zero:
                gain_sb = persist.tile([P, R, D], f32, name="gain_sb")
                bias_sb = persist.tile([P, R, D], f32, name="bias_sb")
                nc.sync.dma_start(
                    gain_sb[:], handles["gain_b"].rearrange("r p d -> p r d"))
                nc.sync.dma_start(
                    bias_sb[:], handles["bias_b"].rearrange("r p d -> p r d"))

            for _rep in range(n_reps):
                # ---------------- prologue: load x, derive hi/lo + transposes
                # fully per-block so block 0's load->hi/lo->transpose chain
                # (the only dependency of the first score matmul) is not
                # queued behind the bulk of the input load
                for jb in range(NB):
                    nc.sync.dma_start(h_nat[0][:, jb, :],
                                      x_d[jb * P:(jb + 1) * P, :])
                    nc.vector.tensor_copy(h_hilo[0][:, jb, 0, :],
                                          h_nat[0][:, jb, :])
                    nc.gpsimd.tensor_tensor(h_hilo[0][:, jb, 1, :],
                                            h_nat[0][:, jb, :],
                                            h_hilo[0][:, jb, 0, :],
                                            op=mybir.AluOpType.subtract)
                    nc.sync.dma_start_transpose(
                        hT_hilo[0][:, :, jb * P:(jb + 1) * P],
                        h_hilo[0][:, jb, :, :])

                # ---------------- rounds (software-pipelined emission)
                # The per-engine instruction streams execute in order, so a
                # message matmul waiting on its GT transpose would block the
                # next block's score matmuls behind it in the PE queue.  Emit
                # the selection front-end (scores/top8/G/GT) one stage ahead
                # of the matmul back-end (messages/epilogue) so the PE always
                # has ready score work while a selection chain completes.
                stage_state = {}

                def emit_front(r, qi):
                    cur = r % 2
                    tt = hT_hilo[cur]
                    W = (qi + 1) * P
                    q_sl = slice(qi * P, (qi + 1) * P)

                    # ---- scores S[:, :W] (3-term bf16 hi/lo)
                    S_sb = work.tile([P, T], f32, tag="S_sb")
                    nchunk = (W + CHUNK - 1) // CHUNK
                    for ch in range(nchunk):
                        c_lo = ch * CHUNK
                        cw = min(CHUNK, W - c_lo)
                        ps = psum_s.tile([P, CHUNK], f32, tag="ps")
                        # (hi,hi), (hi,lo), (lo,hi); hi = rows 0:KT, lo =
                        # rows KT:2KT of hT_hilo
                        terms = ((0, 0), (0, KT), (KT, 0))
                        n_mm = len(terms) * KT
                        i_mm = 0
                        for (o_q, o_k) in terms:
                            for kt in range(KT):
                                nc.tensor.matmul(
                                    ps[:, :cw],
                                    tt[:, o_q + kt, q_sl],
                                    tt[:, o_k + kt, c_lo:c_lo + cw],
                                    start=(i_mm == 0),
                                    stop=(i_mm == n_mm - 1))
                                i_mm += 1
                        if ch % 2 == 0:
                            nc.scalar.copy(S_sb[:, c_lo:c_lo + cw], ps[:, :cw])
                        else:
                            nc.vector.tensor_copy(S_sb[:, c_lo:c_lo + cw],
                                                  ps[:, :cw])

                    # causal mask on diagonal block
                    nc.vector.tensor_tensor(
                        S_sb[:, q_sl], S_sb[:, q_sl], cmask[:],
                        op=mybir.AluOpType.add)

                    # ---- top-8 threshold
                    m8 = small.tile([P, 8], f32, tag="m8")
                    nc.vector.max(out=m8[:], in_=S_sb[:, :W])
                    t8p = small.tile([P, 1], f32, tag="t8p")
                    nc.vector.tensor_scalar(
                        t8p[:], m8[:, 7:8], THRESH_FLOOR, None,
                        op0=mybir.AluOpType.max)

                    # ---- selection mask G (0/1 in bf16)
                    G = work.tile([P, T], bf16, tag="G")
                    nc.gpsimd.tensor_scalar(
                        G[:, :W], S_sb[:, :W], t8p[:], None,
                        op0=mybir.AluOpType.is_ge)

                    gt_all = gtp.tile([P, qi + 1, P], bf16, tag="GT")
                    nc.sync.dma_start_transpose(gt_all[:], G[:, :W])
                    stage_state[(r, qi)] = gt_all

                def emit_back(r, qi):
                    cur, nxt = r % 2, (r + 1) % 2
                    hn_cur = h_nat[r]       # 0,1,2 (keep x intact in h_nat[0])
                    hn_nxt = h_nat[r + 1] if r < R - 1 else h_nat[2]
                    last_round = (r == R - 1)
                    nh = h_hilo[cur]
                    q_sl = slice(qi * P, (qi + 1) * P)
                    gt_all = stage_state.pop((r, qi))

                    # ---- message matmul: msg_raw + alpha*h  (PSUM accum)
                    mp = psum_m.tile([P, D], f32, tag="mp")
                    for jb in range(qi + 1):
                        nc.tensor.matmul(mp[:], gt_all[:, jb, :],
                                         nh[:, jb, 0, :],
                                         start=(jb == 0), stop=False)
                        nc.tensor.matmul(mp[:], gt_all[:, jb, :],
                                         nh[:, jb, 1, :],
                                         start=False, stop=False)
                    pre = "alpha0" if qi == 0 else "alpha8"
                    a_hi = cal[f"{pre}_hi_{r}"]
                    a_lo = cal[f"{pre}_lo_{r}"]
                    nc.tensor.matmul(mp[:], a_hi[:], nh[:, qi, 0, :],
                                     start=False, stop=False)
                    nc.tensor.matmul(mp[:], a_hi[:], nh[:, qi, 1, :],
                                     start=False, stop=alpha_lo_zero[r])
                    if not alpha_lo_zero[r]:
                        nc.tensor.matmul(mp[:], a_lo[:], nh[:, qi, 0, :],
                                         start=False, stop=True)

                    # ---- epilogue
                    sv = cal[f"svec0_{r}" if qi == 0 else f"svec8_{r}"]
                    hnew = ep.tile([P, D], f32, tag="hnew")
                    if gain_is_one and bias_is_zero and not gelu_via_erf:
                        # hnew = gelu(mp * s_i)
                        nc.scalar.activation(
                            hnew[:], mp[:],
                            mybir.ActivationFunctionType.Gelu, scale=sv[:])
                    else:
                        pre_t = ep.tile([P, D], f32, tag="pre_t")
                        nc.scalar.activation(
                            pre_t[:], mp[:],
                            mybir.ActivationFunctionType.Copy, scale=sv[:])
                        if not gain_is_one:
                            nc.vector.tensor_tensor(
                                pre_t[:], pre_t[:], gain_sb[:, r, :],
                                op=mybir.AluOpType.mult)
                        if not bias_is_zero:
                            nc.vector.tensor_tensor(
                                pre_t[:], pre_t[:], bias_sb[:, r, :],
                                op=mybir.AluOpType.add)
                        if gelu_via_erf:
                            erf_t = ep.tile([P, D], f32, tag="erf_t")
                            nc.scalar.activation(
                                erf_t[:], pre_t[:],
                                mybir.ActivationFunctionType.Erf,
                                scale=float(1.0 / np.sqrt(2.0)))
                            nc.vector.tensor_scalar(
                                erf_t[:], erf_t[:], 0.5, 0.5,
                                op0=mybir.AluOpType.mult,
                                op1=mybir.AluOpType.add)
                            nc.vector.tensor_tensor(
                                hnew[:], pre_t[:], erf_t[:],
                                op=mybir.AluOpType.mult)
                        else:
                            nc.scalar.activation(
                                hnew[:], pre_t[:],
                                mybir.ActivationFunctionType.Gelu)
                    # h_next = mom*h + (1-mom)*hnew
                    nc.scalar.mul(hnew[:], hnew[:], 1.0 - momentum)
                    tm = ep.tile([P, D], f32, tag="tm")
                    nc.gpsimd.tensor_scalar(
                        tm[:], hn_cur[:, qi, :], momentum, None,
                        op0=mybir.AluOpType.mult)
                    nc.vector.tensor_tensor(
                        hn_nxt[:, qi, :], tm[:], hnew[:],
                        op=mybir.AluOpType.add)

                    if not last_round:
                        nc.vector.tensor_copy(h_hilo[nxt][:, qi, 0, :],
                                              hn_nxt[:, qi, :])
                        nc.gpsimd.tensor_tensor(
                            h_hilo[nxt][:, qi, 1, :], hn_nxt[:, qi, :],
                            h_hilo[nxt][:, qi, 0, :],
                            op=mybir.AluOpType.subtract)
                        nc.sync.dma_start_transpose(
                            hT_hilo[nxt][:, :, q_sl],
                            h_hilo[nxt][:, qi, :, :])

                LAG = 1
                n_stages = R * NB
                for idx in range(n_stages + LAG):
                    if idx < n_stages:
                        emit_front(*divmod(idx, NB))
                    if idx >= LAG:
                        emit_back(*divmod(idx - LAG, NB))

                # ---------------- output: (h3 - x) * scale
                og = out_d.rearrange("(n p) d -> p n d", p=P)
                for jb in range(NB):
                    od = ep.tile([P, D], f32, tag="od")
                    nc.vector.tensor_tensor(
                        od[:], h_nat[2][:, jb, :], h_nat[0][:, jb, :],
                        op=mybir.AluOpType.subtract)
                    nc.scalar.mul(od[:], od[:], scale)
                    nc.sync.dma_start(og[:, jb, :], od[:])

    return nc


_CACHED = {}


def _get_program(key, *args, **kwargs):
    if key not in _CACHED:
        _CACHED[key] = build_program(*args, **kwargs)
    return _CACHED[key]


def kernel(x, gain, bias, log_mix, log_momentum, log_scale, _trace=False):
    x = np.ascontiguousarray(np.asarray(x, dtype=np.float32))
    gain = np.asarray(gain, dtype=np.float32)
    bias = np.asarray(bias, dtype=np.float32)
    mix = [_sigmoid(v) for v in np.asarray(log_mix, dtype=np.float32)]
    momentum = _sigmoid(np.asarray(log_momentum, dtype=np.float32))
    scale = _softplus(np.asarray(log_scale, dtype=np.float32)) + 0.01

    key = (tuple(np.round(mix, 12)), round(float(momentum), 12),
           round(float(scale), 12),
           gain.tobytes(), bias.tobytes())
    nc = _get_program(key, mix, momentum, scale, gain, bias)

    n_cores = 8
    in_maps = [{"x": x[c % B]} for c in range(n_cores)]
    res = run_bass_kernel_spmd(nc, in_maps, core_ids=list(range(n_cores)),
                               trace=_trace)
    out = np.stack([res.results[b]["out"] for b in range(B)], axis=0)
    if _trace:
        kernel.last_exec_time_ns = res.exec_time_ns
        kernel.last_results = res
    return out

